# revision 43
# baseline (speedup 1.0000x reference)
"""Batched masked dot-product attention on 8 TRN2 NeuronCores.

Problem: query/key/value [16, 2048, 64] f32, valid_len [16] i32.
out = softmax(mask(Q K^T / 8)) V, softmax over the key axis, key positions
>= valid_len[b] masked out.

Device strategy (data parallel over batch fragments, no cross-core comm):
  - Host pre-transposes Q, K to [64, 2048] (d on partitions) so the device
    needs no transposes: the TensorE contraction dim is d for QK^T.
  - Scores are computed TRANSPOSED: s^T[k, q] = (K^T-slice).T @ Q^T, so the
    softmax key axis lands on PSUM partitions. Since the contraction dim is
    only 64, TWO k-tiles run concurrently in PE row groups (tile_position
    (0,0) / (64,0)); K^T is host-packed two-tiles-per-128-partitions and Q^T
    is duplicated into both partition halves. Each pair writes one
    [128, 1024] PSUM tensor (tile-even cols 0-511, tile-odd 512-1023) so
    both MM1s are co-schedulable and one Exp covers the pair.
  - No max-subtraction: |scores/8| <= ~7 for these magnitudes, exp is safe
    in f32 (reference subtracts the row max, which is mathematically a
    no-op). This also makes the unnormalized accumulators ADDITIVE over k,
    so a batch's key range can be split into fragments computed on
    different cores; the host just sums the partial results.
  - Masking is folded into V on the host: V' = [V*m, m, 0] (66 cols, bf16).
    The extra mask column makes the PV matmul also produce the softmax
    denominator Z, and zeroed V rows kill masked keys in the numerator.
    Fully-masked k-tiles are never computed at all.
  - Per k-tile pair and 512-wide q-group: 2 MM1s (bf16), one Exp (scale=1/8
    fused) to bf16 SBUF, then 4 small MM2s per tile (lhsT = exp slice
    [128,128], moving = V' [128,66], bf16) accumulating out[q,c] in PSUM.
    Four 66-col accumulators live at 128-col pitch in one PSUM bank; only
    the first matmul in the bank may set start=True (start clears the whole
    bank's has_written bits).
  - Accumulators are packed tightly by DVE into [128, 4*66] and DMA'd out;
    the host slices, sums fragments per batch, divides by Z, reassembles.
  - Startup: each slot's first k-tile block and first q chunk ship as ONE
    fused DMA ("fc") on the Sync ring so the first matmuls start as early
    as possible after the fixed Tile preamble.

Load balance: all cores run an IDENTICAL kernel with up to 3 k-tile slots of
sizes (A, B, C) chosen per run by an exact DP so that every batch's valid
k-tiles are covered by assigned slots. For uniform-random valid_len this
halves the work vs. the dense kernel while keeping a single NEFF.
"""

from functools import lru_cache
from itertools import product

import numpy as np
import ml_dtypes

import concourse.bass as bass  # noqa: F401  (bass types used via tile/bacc)
import concourse.mybir as mybir
import concourse.tile as tile
from concourse import bacc
from concourse.bass_utils import run_bass_kernel_spmd

B, L, D = 16, 2048, 64
N_CORES = 8
P = 128                 # k-tile size (partition dim)
N_KTILES = L // P       # 16
N_KBLK = N_KTILES // 2  # max packed K^T column blocks per slot
VC = 66                 # V' columns: 64 V + 1 mask + 1 pad
QG = 512                # q-group (one PSUM bank of MM2 accumulators)
N_QG = L // QG          # 4
QB = 128                # MM2 q-block (output partitions)
NQB = QG // QB          # 4 accumulators per q-group
OTW = NQB * VC          # packed output row: 264
SCALE = 1.0 / np.sqrt(np.float32(D))
F32 = mybir.dt.float32
BF16 = mybir.dt.bfloat16
NP_BF16 = ml_dtypes.bfloat16

# Results of the last run_bass_kernel_spmd call (exec_time_ns etc. when
# BASS_TRACE=1); ignored by the grader, used by test.py.
LAST_RESULTS = None


def _ensure_ntff_hook():
    """bass_utils imports antenv.axon_hooks for NTFF tracing under axon; some
    images lack that module. Provide it, wired to the boot shim's hook, so
    BASS_TRACE=1 works. No-op when the real module exists."""
    try:
        import antenv.axon_hooks  # noqa: F401
        return
    except ImportError:
        pass
    import sys
    import types

    import antenv

    mod = types.ModuleType("antenv.axon_hooks")
    holder = [None]
    mod.set_axon_ntff_profile_hook = lambda h: holder.__setitem__(0, h)
    mod.get_axon_ntff_profile_hook = lambda: holder[0]
    sys.modules["antenv.axon_hooks"] = mod
    antenv.axon_hooks = mod
    try:
        from trn_agent_boot.trn_boot import _ntff_profile_via_ctypes

        holder[0] = _ntff_profile_via_ctypes("/opt/axon/libaxon_pjrt.so")
    except Exception:
        pass


# --------------------------------------------------------------------------
# slot-shape search: uniform per-core slot sizes (A >= B >= C), batches cut
# into contiguous k-tile fragments, each fragment assigned to one slot.
# --------------------------------------------------------------------------

def _shape_cost(shape):
    # ACT pair-instructions per q-group (odd leftover tile ~0.7 of a pair)
    return sum((n // 2) + 0.7 * (n % 2) for n in shape if n)


def _solve_shape(need):
    nb = len(need)
    order = np.argsort(need)[::-1]
    sneed = tuple(int(need[i]) for i in order)

    def feasible(sizes):
        ns = len(sizes)

        @lru_cache(maxsize=None)
        def dp(i, avail):
            if i == nb:
                return ()
            n = sneed[i]
            maxc = tuple(min(avail[k], -(-n // sizes[k])) for k in range(ns))
            for combo in product(*(range(c + 1) for c in maxc)):
                cap = sum(x * s for x, s in zip(combo, sizes))
                if cap < n:
                    continue
                if any(x > 0 and cap - s >= n for x, s in zip(combo, sizes)):
                    continue  # non-minimal cover
                rest = dp(i + 1, tuple(a - x for a, x in zip(avail, combo)))
                if rest is not None:
                    return (combo,) + rest
            return None

        return dp(0, tuple(N_CORES for _ in sizes))

    # guaranteed fallback: classic largest-with-smallest pairing
    pair_shape = (max(sneed[i] for i in range(N_CORES)),
                  max(sneed[i] for i in range(N_CORES, nb)), 0)

    cands = []
    for a in range(1, N_KTILES + 1):
        for b in range(0, a + 1):
            for c in range(0, b + 1):
                if (a + b + c) * N_CORES < sum(sneed):
                    continue
                cost = _shape_cost((a, b, c))
                if cost < _shape_cost(pair_shape):
                    cands.append((cost, a + b + c, (a, b, c)))
    cands.sort()

    for _, __, shape in cands:
        sizes = tuple(s for s in shape if s > 0)
        sol = feasible(sizes)
        if sol is None:
            continue
        return sizes, sol, order
    sizes = tuple(s for s in pair_shape if s > 0)
    return sizes, feasible(sizes), order


def _assign(need, sizes, sol, order):
    """-> frags[core][slot] = (batch, t0, length) or None."""
    ns = len(sizes)
    free = [list(range(N_CORES)) for _ in range(ns)]
    frags = [[None] * ns for _ in range(N_CORES)]
    for rank, combo in enumerate(sol):
        b = int(order[rank])
        t0, rem = 0, int(need[b])
        picks = []
        for k in range(ns):
            picks.extend([k] * combo[k])
        picks.sort(key=lambda k: -sizes[k])
        for k in picks:
            core = free[k].pop()
            ln = min(rem, sizes[k])
            frags[core][k] = (b, t0, ln)
            t0 += ln
            rem -= ln
        assert rem == 0
    return frags


def _build(sizes):
    """Build the SPMD kernel for per-core slot k-tile counts `sizes`."""
    ns = len(sizes)
    nc = bacc.Bacc("TRN2", target_bir_lowering=False, debug=False,
                   num_devices=N_CORES)
    fc_d = nc.dram_tensor("fc", [ns, P, P + QG], BF16,
                          kind="ExternalInput").ap()
    qt_d = nc.dram_tensor("qt", [ns, P, L], BF16, kind="ExternalInput").ap()
    kt_d = nc.dram_tensor("kt", [ns, P, N_KBLK * P], BF16,
                          kind="ExternalInput").ap()
    vp_d = nc.dram_tensor("vp", [ns, P, N_KTILES * VC], BF16,
                          kind="ExternalInput").ap()
    ot_d = nc.dram_tensor("ot", [ns, N_QG, P, OTW], F32,
                          kind="ExternalOutput").ap()

    with tile.TileContext(nc) as tc:
        with (
            tc.tile_pool(name="io", bufs=3) as io,
            tc.tile_pool(name="pt", bufs=3) as ptp,
            tc.tile_pool(name="ot", bufs=2) as otp,
            tc.tile_pool(name="ps_s", bufs=3, space="PSUM") as pss,
            tc.tile_pool(name="ps_o", bufs=2, space="PSUM") as pso,
        ):
            for s in range(ns):
                nk = sizes[s]
                nblk = (nk + 1) // 2
                qt = io.tile([P, L], BF16, tag="qt")
                kt = io.tile([P, N_KBLK * P], BF16, tag="kt")
                vp = io.tile([P, N_KTILES * VC], BF16, tag="vp")
                fc = io.tile([P, P + QG], BF16, tag="fc")
                # ONE fused critical DMA (kt block 0 | first q chunk) per
                # slot on the Sync HWDGE ring, then V'; bulk remainders ride
                # the Scalar ring so they never block a slot's first matmuls
                nc.sync.dma_start(fc[:], fc_d[s])
                nc.sync.dma_start(vp[:, : nk * VC], vp_d[s, :, : nk * VC])
                if nblk > 1:
                    nc.scalar.dma_start(kt[:, P: nblk * P],
                                        kt_d[s, :, P: nblk * P])
                nc.scalar.dma_start(qt[:, QG:], qt_d[s, :, QG:])
                for qg in range(N_QG):
                    po = pso.tile([P, NQB * QB], F32, tag="po")
                    for j in range(nblk):
                        tiles = [2 * j] + ([2 * j + 1] if 2 * j + 1 < nk
                                           else [])
                        w = len(tiles) * QG
                        ps = pss.tile([P, 2 * QG], F32, tag="ps")
                        for idx, _ in enumerate(tiles):
                            r0, r1 = (0, D) if idx == 0 else (D, P)
                            lhs = (fc[r0:r1, :P] if j == 0
                                   else kt[r0:r1, j * P:(j + 1) * P])
                            rhs = (fc[r0:r1, P:P + QG] if qg == 0
                                   else qt[r0:r1, qg * QG:(qg + 1) * QG])
                            nc.tensor.matmul(
                                ps[:, idx * QG:(idx + 1) * QG],
                                lhs, rhs,
                                tile_position=((r0, 0) if len(tiles) > 1
                                               else None),
                            )
                        pt = ptp.tile([P, 2 * QG], BF16, tag="pt")
                        nc.scalar.activation(
                            pt[:, :w], ps[:, :w],
                            mybir.ActivationFunctionType.Exp,
                            scale=float(SCALE))
                        for idx, t in enumerate(tiles):
                            for qb in range(NQB):
                                # start=True clears the whole bank's
                                # has_written bits: only the first
                                # accumulator in the bank may assert it
                                nc.tensor.matmul(
                                    po[:, qb * QB:qb * QB + VC],
                                    pt[:, idx * QG + qb * QB:
                                       idx * QG + (qb + 1) * QB],
                                    vp[:, t * VC:(t + 1) * VC],
                                    start=(t == 0 and qb == 0),
                                    stop=(t == nk - 1),
                                    skip_group_check=(qb != 0),
                                )
                    ot = otp.tile([P, OTW], F32, tag="ot")
                    final = (s == ns - 1 and qg == N_QG - 1)
                    for qb in range(NQB):
                        nc.vector.tensor_copy(
                            ot[:, qb * VC:(qb + 1) * VC],
                            po[:, qb * QB:qb * QB + VC])
                    if final:
                        # split the last store so its fixed HBM write-receipt
                        # latency starts right after the first two copies
                        nc.sync.dma_start(ot_d[s, qg, :, :2 * VC],
                                          ot[:, :2 * VC])
                        nc.scalar.dma_start(ot_d[s, qg, :, 2 * VC:],
                                            ot[:, 2 * VC:])
                    else:
                        nc.sync.dma_start(ot_d[s, qg], ot[:])
    nc.compile()
    return nc


def kernel(query, key, value, valid_len):
    global LAST_RESULTS
    query = np.asarray(query, np.float32)
    key = np.asarray(key, np.float32)
    value = np.asarray(value, np.float32)
    assert query.shape == (B, L, D) and np.shape(valid_len) == (B,)

    vl = np.clip(np.asarray(valid_len).astype(np.int64), 1, L)
    need = np.maximum(1, -(-vl // P))  # ceil(vl/128), in [1, 16]

    try:
        sizes, sol, order = _solve_shape(tuple(int(n) for n in need))
    except Exception:
        # fall back to the always-feasible big-with-small pairing
        order = np.argsort(need)[::-1]
        sizes = (int(need[order[0]]), int(need[order[N_CORES]]))
        sol = tuple((1, 0) if r < N_CORES else (0, 1)
                    for r in range(B))
    frags = _assign(need, sizes, sol, order)
    ns = len(sizes)

    nc = _build(sizes)

    qts = {}  # batch -> duplicated-row bf16 Q^T
    kts = {}  # batch -> bf16 K^T
    vps = {}  # batch -> [L, VC] f32 masked V'
    for bi in range(B):
        qT = query[bi].T.astype(NP_BF16)
        qts[bi] = np.concatenate([qT, qT], axis=0)  # [128, L]
        kts[bi] = key[bi].T.astype(NP_BF16)         # [64, L]
        m = (np.arange(L) < vl[bi]).astype(np.float32)
        vprime = np.zeros((L, VC), np.float32)
        vprime[:, :D] = value[bi] * m[:, None]
        vprime[:, D] = m
        vps[bi] = vprime

    in_maps = []
    for c in range(N_CORES):
        fc = np.zeros((ns, P, P + QG), NP_BF16)
        qt = np.zeros((ns, P, L), NP_BF16)
        kt = np.zeros((ns, P, N_KBLK * P), NP_BF16)
        vp = np.zeros((ns, P, N_KTILES * VC), NP_BF16)
        for s in range(ns):
            fr = frags[c][s]
            if fr is None:
                continue
            bi, t0, ln = fr
            qt[s] = qts[bi]
            kT = kts[bi]
            for u in range(0, ln, 2):
                blk = u // 2
                ta = t0 + u
                kt[s, :D, blk * P:(blk + 1) * P] = \
                    kT[:, ta * P:(ta + 1) * P]
                if u + 1 < ln:
                    kt[s, D:, blk * P:(blk + 1) * P] = \
                        kT[:, (ta + 1) * P:(ta + 2) * P]
            # fragment V' tiles, swizzled so each partition line is one
            # contiguous DMA segment: [ln, 128, VC] -> [128, ln*VC]
            vfrag = vps[bi][t0 * P:(t0 + ln) * P].reshape(ln, P, VC)
            vp[s, :, :ln * VC] = vfrag.transpose(1, 0, 2).reshape(
                P, ln * VC).astype(NP_BF16)
            fc[s, :, :P] = kt[s, :, :P]
            fc[s, :, P:] = qt[s, :, :QG]
        in_maps.append({"fc": fc, "qt": qt, "kt": kt, "vp": vp})

    _ensure_ntff_hook()
    res = run_bass_kernel_spmd(nc, in_maps, core_ids=list(range(N_CORES)))
    LAST_RESULTS = res

    acc = np.zeros((B, L, VC), np.float64)
    for c in range(N_CORES):
        o = res.results[c]["ot"]  # [ns, N_QG, P, OTW]
        o = o.reshape(ns, N_QG, P, NQB, VC).transpose(0, 1, 3, 2, 4)
        o = o.reshape(ns, L, VC)  # [slot, q, c], q = qg*512 + qb*128 + p
        for s in range(ns):
            fr = frags[c][s]
            if fr is None:
                continue
            acc[fr[0]] += o[s]
    out = (acc[:, :, :D] / acc[:, :, D:D + 1]).astype(np.float32)
    return out
